# revision 6
# baseline (speedup 1.0000x reference)
"""Trainium2 Bass kernel for nn_Attention_84567906058480 — v2.

Multi-head attention (B=4, T=2048, C=1024, H=16, D=64) on 8 NeuronCores.
Core c = (batch c//2, head-group c%2); pairwise ReduceScatter after to_out.

v2 structure (vs v1):
- Score matmuls are K=64 row-tiled PAIRS: head 2fb contracts on partitions
  0-63 (tile_position row 0) and head 2fb+1 on partitions 64-127 (row 64).
  The PE runs both concurrently (disjoint row-groups), halving score time.
  kT keeps the natural [128 feats, T] layout (no zero padding).
- The scalar engine runs ONLY exps ([128,1024] batches); everything else
  is DVE/PE/gpsimd.
- QKV projections, bias broadcast, and output-projection chunks are emitted
  as "filler" groups pumped between attention kp-iterations, so the PE works
  through projection backlog during the scalar-bound attention phase.
- Unit order interleaves q-chunks {0,2} across fb first (window A), then
  {1,3} (window B); output chunks tq0/tq1 overlap window B; tq2/tq3 drain in
  the tail with 4 small ReduceScatters, output written bf16 DRAM->DRAM.
"""

import os
import sys
import types
import contextlib
from collections import deque

import numpy as np

if "/opt/trn_rl_repo" not in sys.path:
    sys.path.insert(0, "/opt/trn_rl_repo")

import ml_dtypes
import concourse.bass as bass  # noqa: F401
import concourse.mybir as mybir
import concourse.tile as tile
from concourse import bacc
from concourse import bass_utils

F32 = mybir.dt.float32
BF16 = mybir.dt.bfloat16
AF = mybir.ActivationFunctionType

B, T, C = 4, 2048, 1024
H, D = 16, 64
HPC = 8
FS = HPC * D       # 512
N_CORES = 8
PAIRS = [[0, 1], [2, 3], [4, 5], [6, 7]]

NT = T // 128      # 16
NCT = C // 128     # 8
NFB = FS // 128    # 4
QW = 512
NQC = T // QW      # 4
NKP = NT // 2      # 8


def _emit(nc, tc, xt_ext, wqt_ext, wkt_ext, wvt_ext, wot_ext, bo_ext, out_ext):
    with tc.tile_pool(name="const", bufs=1) as constp, \
         tc.tile_pool(name="persist", bufs=1) as pp, \
         tc.tile_pool(name="pbc", bufs=1) as pbc, \
         tc.tile_pool(name="pd", bufs=4) as pd, \
         tc.tile_pool(name="pdram", bufs=4, space="DRAM") as pdram, \
         tc.tile_pool(name="ps_sT", bufs=1, space="PSUM") as ps_sT, \
         tc.tile_pool(name="ps_oT", bufs=2, space="PSUM") as ps_oT, \
         tc.tile_pool(name="ps_misc", bufs=2, space="PSUM") as ps_misc:

        # ---- constants -------------------------------------------------
        ones_col = constp.tile([1, 128], F32, tag="ones")
        nc.gpsimd.memset(ones_col[:, :], 1.0)
        Emat = constp.tile([128, 64], BF16, tag="Emat")
        nc.gpsimd.memset(Emat[:, :], 0.0)
        nc.gpsimd.memset(Emat[0:1, :], 1.0)
        bo_row = constp.tile([1, C], F32, tag="bo_row")
        bo_bcast = constp.tile([128, C], F32, tag="bo_bcast")
        l_pad = constp.tile([128, QW], BF16, tag="l_pad")
        nc.gpsimd.memset(l_pad[:, :], 0.0)

        # ---- persistent activation storage (bf16) ----------------------
        qT = [pp.tile([128, T], BF16, tag=f"qT{fb}", name=f"qT{fb}") for fb in range(NFB)]
        kT = [pp.tile([128, T], BF16, tag=f"kT{fb}", name=f"kT{fb}") for fb in range(NFB)]
        v_ext = [pp.tile([128, HPC * 65], BF16, tag=f"vx{tt}", name=f"vx{tt}") for tt in range(NT)]
        woT = [pp.tile([128, C], BF16, tag=f"woT{fb}", name=f"woT{fb}") for fb in range(NFB)]
        lout = [pp.tile([128, T], BF16, tag=f"lo{fb}", name=f"lo{fb}") for fb in range(NFB)]

        # ---- input / weight DMAs (ct-major so projections can stream) --
        xT = [pbc.tile([128, T], BF16, tag=f"xT{ct}", name=f"xT{ct}", bufs=1) for ct in range(NCT)]
        wqTf = pbc.tile([128, NCT * FS], BF16, tag="wqTf", bufs=1)
        wkTf = pbc.tile([128, NCT * FS], BF16, tag="wkTf", bufs=1)
        wvT = pbc.tile([128, NCT * FS], BF16, tag="wvT", bufs=1)
        for ct in range(NCT):
            nc.sync.dma_start(xT[ct][:, :], xt_ext[ct * 128:(ct + 1) * 128, :])
            nc.gpsimd.dma_start(wqTf[:, ct * FS:(ct + 1) * FS],
                                wqt_ext[ct * 128:(ct + 1) * 128, :])
            nc.gpsimd.dma_start(wkTf[:, ct * FS:(ct + 1) * FS],
                                wkt_ext[ct * 128:(ct + 1) * 128, :])
        nc.gpsimd.dma_start(
            wvT[:].rearrange("p (ct f) -> p ct f", f=FS),
            wvt_ext[:].rearrange("(ct p) f -> p ct f", p=128))
        for fb in range(NFB):
            nc.gpsimd.dma_start(woT[fb][:, :], wot_ext[fb * 128:(fb + 1) * 128, :])
        nc.gpsimd.dma_start(bo_row[:, :], bo_ext[:].unsqueeze(0))

        # score PSUM: per head-pair, sT[0] = head 2fb, sT[1] = head 2fb+1
        sTs = [ps_sT.tile([128, 1024], F32, tag=f"sT{i}", name=f"sT{i}", bufs=1)
               for i in range(2)]

        # ---------- filler machinery ------------------------------------
        fillers = deque()
        emitted = set()

        def pump(n=1):
            for _ in range(n):
                if not fillers:
                    return
                key, fn = fillers.popleft()
                fn()
                emitted.add(key)

        def pump_until(key):
            if key in emitted:
                return
            while True:
                if not fillers:
                    raise RuntimeError(f"missing filler {key}")
                k, fn = fillers.popleft()
                fn()
                emitted.add(k)
                if k == key:
                    return

        def g_qk(proj, fb, tch):
            wf = wqTf if proj == "q" else wkTf
            dstT = qT if proj == "q" else kT

            def fn():
                acc = ps_misc.tile([128, QW], F32, tag="misc", name="acc")
                for ct in range(NCT):
                    nc.tensor.matmul(
                        acc[:, :],
                        wf[:, ct * FS + fb * 128: ct * FS + fb * 128 + 128],
                        xT[ct][:, tch * QW:(tch + 1) * QW],
                        start=(ct == 0), stop=(ct == NCT - 1))
                nc.vector.tensor_copy(dstT[fb][:, tch * QW:(tch + 1) * QW], acc[:, :])
            return ((proj, fb, tch), fn)

        def g_v(tt):
            def fn():
                acc = ps_misc.tile([128, FS], F32, tag="misc", name="acc")
                for ct in range(NCT):
                    nc.tensor.matmul(
                        acc[:, :],
                        xT[ct][:, tt * 128:(tt + 1) * 128],
                        wvT[:, ct * FS:(ct + 1) * FS],
                        start=(ct == 0), stop=(ct == NCT - 1))
                nc.gpsimd.memset(v_ext[tt][:, :], 1.0)
                dst = v_ext[tt][:].rearrange("p (h e) -> p h e", e=65)[:, :, 0:64]
                src = acc[:].rearrange("p (h e) -> p h e", e=64)
                nc.vector.tensor_copy(dst, src)
            return (("v", tt), fn)

        def g_bias(cc):
            def fn():
                bb = ps_misc.tile([128, 512], F32, tag="misc", name="acc")
                nc.tensor.matmul(bb[:, :], ones_col[:, :],
                                 bo_row[:, cc * 512:(cc + 1) * 512],
                                 start=True, stop=True)
                nc.vector.tensor_copy(bo_bcast[:, cc * 512:(cc + 1) * 512], bb[:, :])
            return (("bias", cc), fn)

        # ---- output projection chunks + ReduceScatter ------------------
        rs_bufs = {}

        def g_oproj(tq, half, t2, cc):
            # chunk tq covers tokens [tq*256,+256) of each half; rs rows:
            # half0 -> 0-255, half1 -> 256-511
            is_first = (half, t2, cc) == (0, 0, 0)
            is_last = (half, t2, cc) == (1, 1, 1)

            def fn():
                if is_first:
                    rs_bufs[tq] = (
                        pdram.tile([512, C], BF16, tag="rs_in", name=f"rs_in{tq}"),
                        pdram.tile([256, C], BF16, tag="rs_out", name=f"rs_out{tq}"))
                rs_in, rs_out = rs_bufs[tq]
                tok0 = half * 1024 + tq * 256 + t2 * 128
                pj = ps_misc.tile([128, 512], F32, tag="misc", name="pj")
                for fb in range(NFB):
                    nc.tensor.matmul(
                        pj[:, :],
                        lout[fb][:, tok0:tok0 + 128],
                        woT[fb][:, cc * 512:(cc + 1) * 512],
                        start=(fb == 0), stop=(fb == NFB - 1))
                ot = pd.tile([128, 512], BF16, tag="ot")
                nc.vector.tensor_add(ot[:, :], pj[:, :],
                                     bo_bcast[:, cc * 512:(cc + 1) * 512])
                r0 = half * 256 + t2 * 128
                nc.sync.dma_start(
                    rs_in[r0:r0 + 128, cc * 512:(cc + 1) * 512], ot[:, :])
                if is_last:
                    nc.gpsimd.collective_compute(
                        "ReduceScatter", mybir.AluOpType.add,
                        replica_groups=PAIRS,
                        ins=[rs_in.opt()], outs=[rs_out.opt()])
                    for dr in range(2):
                        rs_sb = pd.tile([128, C], BF16, tag="rs_sb")
                        nc.sync.dma_start(rs_sb[:, :],
                                          rs_out[dr * 128:(dr + 1) * 128, :])
                        r0 = tq * 256 + dr * 128
                        nc.sync.dma_start(out_ext[r0:r0 + 128, :], rs_sb[:, :])
            return (("oproj", tq, half, t2, cc), fn)

        def enqueue_oproj(tq, halves=(0, 1)):
            for half in halves:
                for t2 in range(2):
                    for cc in range(2):
                        fillers.append(g_oproj(tq, half, t2, cc))

        # ---------- attention unit: one head-pair, one q chunk ----------
        pending_norm = []

        def attn_pair(fb, qc, lag, on_kp=None):
            h0, h1 = 2 * fb, 2 * fb + 1
            q0 = qT[fb][0:64, qc * QW:(qc + 1) * QW]
            q1 = qT[fb][64:128, qc * QW:(qc + 1) * QW]
            o = [ps_oT.tile([65, QW], F32, tag="outT", name=f"outT{i}")
                 for i in range(2)]
            pTs = {}

            def emit_outT(kp):
                pump_until(("v", 2 * kp + 1))
                for hh, h in ((0, h0), (1, h1)):
                    for j in range(2):
                        kt = kp * 2 + j
                        nc.tensor.matmul(
                            o[hh][:, :],
                            v_ext[kt][:, h * 65:(h + 1) * 65],
                            pTs[(hh, kp)][:, j * 512:(j + 1) * 512],
                            start=(kp == 0 and j == 0),
                            stop=(kp == NKP - 1 and j == 1))
                for hh in range(2):
                    del pTs[(hh, kp)]

            for kp in range(NKP):
                pump_until(("k", fb, kp // 2))
                pump_until(("q", fb, qc))
                for j in range(2):
                    kt = kp * 2 + j
                    # row-tiled concurrent pair: head h0 on partitions 0-63,
                    # head h1 on 64-127
                    nc.tensor.matmul(
                        sTs[0][:, j * 512:(j + 1) * 512],
                        kT[fb][0:64, kt * 128:(kt + 1) * 128],
                        q0, start=True, stop=True)
                    nc.tensor.matmul(
                        sTs[1][:, j * 512:(j + 1) * 512],
                        kT[fb][64:128, kt * 128:(kt + 1) * 128],
                        q1, start=True, stop=True)
                for hh in range(2):
                    pT = pd.tile([128, 1024], BF16, tag="pT", bufs=20)
                    nc.scalar.activation(pT[:, :], sTs[hh][:, :], AF.Exp)
                    pTs[(hh, kp)] = pT
                if kp == 1:
                    while pending_norm:
                        pending_norm.pop(0)()
                if on_kp is not None:
                    on_kp(kp)
                pump(2 if unit_idx[0] == 1 else 1)
                if kp >= lag:
                    emit_outT(kp - lag)
            for kp in range(NKP - lag, NKP):
                emit_outT(kp)

            def mk_norm(hh):
                outT = o[hh]

                def norm():
                    # broadcast denominators across 64 partitions via the
                    # one-hot-row matmul, then partition-parallel reciprocal
                    nc.vector.tensor_copy(l_pad[0:1, :], outT[64:65, :])
                    rb_ps = ps_misc.tile([128, QW], F32, tag="misc", name="rb_ps")
                    nc.tensor.matmul(rb_ps[0:64, :], Emat[:, :], l_pad[:, :],
                                     start=True, stop=True)
                    rb = pd.tile([64, QW], F32, tag="rb_sb")
                    nc.vector.reciprocal_approx_fast(rb[:, :], rb_ps[0:64, :])
                    nc.vector.tensor_mul(
                        lout[fb][hh * 64:(hh + 1) * 64, qc * QW:(qc + 1) * QW],
                        outT[0:64, :], rb[:, :])
                return norm
            pending_norm.append(mk_norm(0))
            pending_norm.append(mk_norm(1))

        # ---------- schedule --------------------------------------------
        # filler queue: k/q for fb0 first (prefix of unit 1), then all v
        # (unit 1's attnV tail needs them), then the remaining fb's k/q
        fillers.append(g_qk("k", 0, 0))
        fillers.append(g_qk("q", 0, 0))
        fillers.append(g_qk("k", 0, 1))
        fillers.append(g_qk("k", 0, 2))
        fillers.append(g_qk("k", 0, 3))
        fillers.append(g_qk("q", 0, 2))
        for tt in range(NT):
            fillers.append(g_v(tt))
        for fb in range(1, NFB):
            for tch in range(NQC):
                fillers.append(g_qk("k", fb, tch))
            fillers.append(g_qk("q", fb, 0))
            fillers.append(g_qk("q", fb, 2))
        fillers.append(g_bias(0))
        fillers.append(g_bias(1))

        unit_idx = [0]

        def run_unit(fb, qc):
            unit_idx[0] += 1
            idx = unit_idx[0]
            lag = 8 if idx == 1 else 2

            def on_kp(kp):
                if kp == 2 and idx == 9:
                    # window B begun: qc0+qc2 lout complete (norms of unit 8
                    # drained at kp1) -> queue output chunks 0,1
                    enqueue_oproj(0)
                    enqueue_oproj(1)
                if kp == 2 and idx == 13:
                    # qc1 lout complete -> half0 of the tail chunks
                    enqueue_oproj(2, halves=(0,))
                    enqueue_oproj(3, halves=(0,))
            attn_pair(fb, qc, lag, on_kp)

        # window A: q-chunks 0 and 2, fb-major
        for fb in range(NFB):
            run_unit(fb, 0)
            run_unit(fb, 2)
        for fb in range(NFB):
            fillers.append(g_qk("q", fb, 1))
            fillers.append(g_qk("q", fb, 3))
        # window B: all of qc1 first, then qc3, so qc1-dependent output
        # chunk halves can drain before the last unit
        for fb in range(NFB):
            run_unit(fb, 1)
        for fb in range(NFB):
            run_unit(fb, 3)
        while pending_norm:
            pending_norm.pop(0)()
        enqueue_oproj(2, halves=(1,))
        enqueue_oproj(3, halves=(1,))
        while fillers:
            pump(1)


def _build_nc():
    nc = bacc.Bacc("TRN2", target_bir_lowering=False, debug=False,
                   num_devices=N_CORES)
    xt_ext = nc.dram_tensor("xt", [C, T], BF16, kind="ExternalInput")
    wqt_ext = nc.dram_tensor("wqt", [C, FS], BF16, kind="ExternalInput")
    wkt_ext = nc.dram_tensor("wkt", [C, FS], BF16, kind="ExternalInput")
    wvt_ext = nc.dram_tensor("wvt", [C, FS], BF16, kind="ExternalInput")
    wot_ext = nc.dram_tensor("wot", [FS, C], BF16, kind="ExternalInput")
    bo_ext = nc.dram_tensor("bo", [C], F32, kind="ExternalInput")
    out_ext = nc.dram_tensor("out", [T // 2, C], BF16, kind="ExternalOutput")
    with tile.TileContext(nc) as tc:
        _emit(nc, tc, xt_ext, wqt_ext, wkt_ext, wvt_ext, wot_ext, bo_ext, out_ext)
    nc.finalize()
    return nc


# ---------------------------------------------------------------------------
# NTFF profiling under axon (used when KERNEL_TRACE=1)
# ---------------------------------------------------------------------------
def _ensure_axon_hooks():
    try:
        from antenv.axon_hooks import get_axon_ntff_profile_hook  # noqa: F401
        return
    except ImportError:
        pass
    import ctypes
    import antenv

    so_path = "/opt/axon/libaxon_pjrt.so"
    lib = ctypes.CDLL(so_path)
    if not hasattr(lib, "axon_start_nrt_profile"):
        return
    lib.axon_start_nrt_profile.argtypes = [ctypes.POINTER(ctypes.c_int64),
                                           ctypes.c_size_t]
    lib.axon_start_nrt_profile.restype = ctypes.c_int64
    lib.axon_stop_nrt_profile.argtypes = [ctypes.c_char_p]
    lib.axon_stop_nrt_profile.restype = ctypes.c_int64

    @contextlib.contextmanager
    def _hook(output_dir, device_ids):
        import jax
        jax.devices()
        if device_ids:
            ids = (ctypes.c_int64 * len(device_ids))(*device_ids)
            rc = lib.axon_start_nrt_profile(ids, len(device_ids))
        else:
            rc = lib.axon_start_nrt_profile(None, 0)
        if rc != 0:
            raise RuntimeError(f"axon_start_nrt_profile rc={rc}")
        try:
            yield
        finally:
            n = lib.axon_stop_nrt_profile(str(output_dir).encode())
            print(f"ntff profile: {n} file(s) -> {output_dir}", file=sys.stderr)

    holder = [_hook]
    mod = types.ModuleType("antenv.axon_hooks")
    mod.get_axon_ntff_profile_hook = lambda: holder[0]
    mod.set_axon_ntff_profile_hook = lambda h: holder.__setitem__(0, h)
    sys.modules["antenv.axon_hooks"] = mod
    antenv.axon_hooks = mod
    bass_utils.upload_artifacts = lambda tmpdir: f"(local:{tmpdir})"


_NC = None
LAST = {}


def kernel(hidden_states, wq, wk, wv, wo, bo):
    global _NC
    hidden_states = np.asarray(hidden_states, dtype=np.float32)
    wq = np.asarray(wq, dtype=np.float32)
    wk = np.asarray(wk, dtype=np.float32)
    wv = np.asarray(wv, dtype=np.float32)
    wo = np.asarray(wo, dtype=np.float32)
    bo = np.asarray(bo, dtype=np.float32)

    if _NC is None:
        _NC = _build_nc()

    bf = ml_dtypes.bfloat16
    scale = np.float32(D ** -0.5)
    in_maps = []
    for c in range(N_CORES):
        b, hg = divmod(c, 2)
        fr = hg * FS
        in_maps.append({
            "xt": np.ascontiguousarray(hidden_states[b].T).astype(bf),
            "wqt": np.ascontiguousarray((wq[fr:fr + FS] * scale).T).astype(bf),
            "wkt": np.ascontiguousarray(wk[fr:fr + FS].T).astype(bf),
            "wvt": np.ascontiguousarray(wv[fr:fr + FS].T).astype(bf),
            "wot": np.ascontiguousarray(wo[:, fr:fr + FS].T).astype(bf),
            "bo": bo * np.float32(0.5),
        })

    trace = os.environ.get("KERNEL_TRACE", "0") == "1"
    if trace:
        _ensure_axon_hooks()
    res = bass_utils.run_bass_kernel_spmd(
        _NC, in_maps, core_ids=list(range(N_CORES)), trace=trace)
    LAST["exec_time_ns"] = res.exec_time_ns
    LAST["res"] = res

    y = np.empty((B, T, C), dtype=np.float32)
    for c in range(N_CORES):
        b, hg = divmod(c, 2)
        y[b, hg * (T // 2):(hg + 1) * (T // 2), :] = res.results[c]["out"]
    return y
